# revision 28
# baseline (speedup 1.0000x reference)
import sys

if "/opt/trn_rl_repo" not in sys.path:
    sys.path.insert(0, "/opt/trn_rl_repo")

from contextlib import ExitStack

import numpy as np

from concourse import bacc, mybir, tile
from concourse.bass_utils import run_bass_kernel_spmd

f32 = mybir.dt.float32
bf16 = mybir.dt.bfloat16
Alu = mybir.AluOpType
Act = mybir.ActivationFunctionType

P = 128
C_MAGIC = 1.5 * 2 ** 23
QDIV = 127.5 * (1.0 - 2.0 ** -20)
INV_QDIV = 1.0 / QDIV
TINY = 1e-30

M, K, N = 8192, 4096, 4096
MG, NG = 2, 4
M_loc, N_loc = M // MG, N // NG
N_CORES = MG * NG

HEAD_MIS = 5


def build_aqt(nc, M_loc, K, N_loc, W=512):
    KT, MT = K // P, M_loc // P
    NB = N_loc // W
    NT = N_loc // P
    TPB = NT // NB
    H = K // 2
    HT = H // P

    lhs = nc.declare_dram_parameter("lhs", [M_loc, K], f32, isOutput=False)
    rhsT = nc.declare_dram_parameter("rhsT", [N_loc, K], f32, isOutput=False)
    out = nc.declare_dram_parameter("out", [M_loc, N_loc], f32, isOutput=True)

    with tile.TileContext(nc) as tc, ExitStack() as ctx:
        pool = lambda name, bufs: ctx.enter_context(tc.tile_pool(name=name, bufs=bufs))
        qr_pool = pool("qr", 1)
        raws = pool("raws", 8)
        rqf = pool("rqf", 3)
        rsc = pool("rsc", 4)
        lqb = pool("lqb", 2)
        lqt = pool("lqt", 5)
        lsc = pool("lsc", 1)
        sml = pool("sml", 8)
        opool = pool("o1", 3)
        psum = ctx.enter_context(tc.tile_pool(name="psum", bufs=6, space="PSUM"))

        s_l_all = lsc.tile([P, MT], f32)
        qr_nb = [qr_pool.tile([P, KT, W], bf16, name=f"qrnb{nb}")
                 for nb in range(NB)]

        rraw_t, ram_t = {}, {}

        def emit_rhs_load(j):
            hs = []
            for h in range(2):
                raw = raws.tile([P, H], f32, name="raw")
                nc.gpsimd.dma_start(raw[:], rhsT[j * P:(j + 1) * P, h * H:(h + 1) * H])
                hs.append(raw)
            rraw_t[j] = hs

        def emit_rhs_red(j):
            ams = []
            for h in range(2):
                am = sml.tile([P, 1], f32, name="ram")
                nc.vector.tensor_reduce(am[:], rraw_t[j][h][:],
                                        axis=mybir.AxisListType.X,
                                        op=Alu.max, apply_absolute_value=True)
                ams.append(am)
            ram_t[j] = ams

        def emit_rhs_quant(j):
            nb, jo = divmod(j, TPB)
            hs = rraw_t.pop(j)
            am0, am1 = ram_t.pop(j)
            nc.vector.tensor_tensor(am0[:], am0[:], am1[:], op=Alu.max)
            s_col = rsc.tile([P, 1], f32, name="rs")
            nc.vector.tensor_scalar(s_col[:], am0[:], TINY, INV_QDIV,
                                    op0=Alu.max, op1=Alu.mult)
            r_col = sml.tile([P, 1], f32, name="rr")
            nc.vector.reciprocal(r_col[:], s_col[:])
            for h in range(2):
                raw = hs[h]
                nc.scalar.activation(raw[:], raw[:], Act.Copy,
                                     bias=C_MAGIC, scale=r_col[:])
                qf = rqf.tile([P, H], bf16, name="rqf")
                nc.gpsimd.tensor_scalar(qf[:], raw[:], C_MAGIC, s_col[:],
                                        op0=Alu.subtract, op1=Alu.mult)
                nc.sync.dma_start_transpose(
                    qr_nb[nb][:, h * HT:(h + 1) * HT, jo * P:(jo + 1) * P], qf[:])

        lraw_t, lqt_t = {}, {}

        def emit_lhs_load(mi):
            rs = slice(mi * P, (mi + 1) * P)
            hs = []
            for h in range(2):
                raw = raws.tile([P, H], f32, name="raw")
                nc.gpsimd.dma_start(raw[:], lhs[rs, h * H:(h + 1) * H])
                hs.append(raw)
            lraw_t[mi] = hs

        def lq(mi):
            hs = lraw_t.pop(mi)
            ams = []
            for h in range(2):
                am = sml.tile([P, 1], f32, name="lam")
                nc.vector.tensor_reduce(am[:], hs[h][:], axis=mybir.AxisListType.X,
                                        op=Alu.max, apply_absolute_value=True)
                ams.append(am)
            am0, am1 = ams
            nc.vector.tensor_tensor(am0[:], am0[:], am1[:], op=Alu.max)
            s_col = s_l_all[:, mi:mi + 1]
            nc.vector.tensor_scalar(s_col, am0[:], TINY, INV_QDIV,
                                    op0=Alu.max, op1=Alu.mult)
            r_col = sml.tile([P, 1], f32, name="lr")
            nc.vector.reciprocal(r_col[:], s_col)
            qb = lqb.tile([P, K], bf16, name="lqb")
            for h in range(2):
                raw = hs[h]
                if mi >= 8:
                    nc.vector.tensor_scalar(raw[:], raw[:], r_col[:], C_MAGIC,
                                            op0=Alu.mult, op1=Alu.add)
                else:
                    nc.scalar.activation(raw[:], raw[:], Act.Copy,
                                         bias=C_MAGIC, scale=r_col[:])
                nc.vector.tensor_scalar(qb[:, h * H:(h + 1) * H], raw[:],
                                        C_MAGIC, None, op0=Alu.subtract)
            qt = lqt.tile([P, KT, P], bf16, name="lqt")
            nc.sync.dma_start_transpose(qt[:, :, :], qb[:, :])
            lqt_t[mi] = qt

        def emit_mm_group(mi, nb, last):
            qt = lqt_t[mi]
            if last:
                del lqt_t[mi]
            rs = slice(mi * P, (mi + 1) * P)
            ps = psum.tile([P, W], f32, name="ps")
            for kt in range(KT):
                nc.tensor.matmul(ps[:], qt[:, kt, :], qr_nb[nb][:, kt, :],
                                 start=(kt == 0), stop=(kt == KT - 1))
            o1 = opool.tile([P, W], f32, name="o1")
            nc.scalar.activation(o1[:], ps[:], Act.Copy, bias=0.0,
                                 scale=s_l_all[:, mi:mi + 1])
            nc.sync.dma_start(out[rs, nb * W:(nb + 1) * W], o1[:])

        emit_lhs_load(0)
        emit_rhs_load(0)
        emit_rhs_load(1)
        emit_rhs_load(2)
        emit_rhs_red(0)
        emit_rhs_quant(0)
        lq(0)
        emit_rhs_load(3)
        emit_rhs_red(1)
        emit_rhs_quant(1)
        emit_lhs_load(1)
        emit_rhs_red(2)
        emit_rhs_quant(2)
        emit_lhs_load(2)
        emit_rhs_load(4)
        emit_rhs_red(3)
        emit_rhs_quant(3)
        lq(1)
        emit_rhs_load(5)
        lq(2)
        emit_lhs_load(3)
        emit_rhs_red(4)
        emit_rhs_quant(4)
        emit_lhs_load(4)
        emit_rhs_load(6)
        lq(3)
        emit_rhs_red(5)
        emit_rhs_quant(5)
        emit_rhs_load(7)
        lq(4)
        emit_rhs_red(6)
        emit_rhs_quant(6)
        emit_lhs_load(5)
        emit_rhs_red(7)
        emit_rhs_quant(7)
        emit_lhs_load(6)
        for mi in range(HEAD_MIS):
            emit_mm_group(mi, 0, last=False)
        emit_mm_group(0, 1, last=True)
        lq(5)
        emit_lhs_load(7)
        emit_mm_group(1, 1, last=True)
        lq(6)
        emit_lhs_load(8)
        for mi in range(2, HEAD_MIS):
            emit_mm_group(mi, 1, last=True)

        for mi in range(HEAD_MIS, MT):
            emit_mm_group(mi, 0, last=False)
            if mi + 4 < MT:
                emit_lhs_load(mi + 4)
            if mi + 2 < MT:
                lq(mi + 2)
            emit_mm_group(mi, 1, last=True)
    return nc


_COMPILED_NC = None


def _get_compiled():
    global _COMPILED_NC
    if _COMPILED_NC is None:
        nc = bacc.Bacc("TRN2", target_bir_lowering=False, debug=False,
                       num_devices=N_CORES)
        build_aqt(nc, M_loc, K, N_loc)
        nc.compile()
        _COMPILED_NC = nc
    return _COMPILED_NC


def _shard(lhs, rhs):
    rhsT = np.ascontiguousarray(rhs.T)
    in_maps = []
    for i in range(N_CORES):
        mg, ng = divmod(i, NG)
        in_maps.append({
            "lhs": np.ascontiguousarray(lhs[mg * M_loc:(mg + 1) * M_loc, :]),
            "rhsT": rhsT[ng * N_loc:(ng + 1) * N_loc, :],
        })
    return in_maps


def kernel(lhs, rhs, _trace=False, _trace_kwargs=None):
    lhs = np.asarray(lhs, np.float32)
    rhs = np.asarray(rhs, np.float32)
    nc = _get_compiled()
    res = run_bass_kernel_spmd(nc, _shard(lhs, rhs), core_ids=list(range(N_CORES)),
                               trace=_trace, **(_trace_kwargs or {}))
    out = np.empty((M, N), np.float32)
    for i in range(N_CORES):
        mg, ng = divmod(i, NG)
        out[mg * M_loc:(mg + 1) * M_loc, ng * N_loc:(ng + 1) * N_loc] = \
            res.results[i]["out"]
    kernel.last_result = res
    return out


# revision 30
# speedup vs baseline: 1.4993x; 1.4993x over previous
import sys

if "/opt/trn_rl_repo" not in sys.path:
    sys.path.insert(0, "/opt/trn_rl_repo")

from contextlib import ExitStack

import numpy as np

from concourse import bacc, mybir, tile
from concourse.bass_utils import run_bass_kernel_spmd

f32 = mybir.dt.float32
bf16 = mybir.dt.bfloat16
Alu = mybir.AluOpType
Act = mybir.ActivationFunctionType

P = 128
C_MAGIC = 1.5 * 2 ** 23
QDIV = 127.5 * (1.0 - 2.0 ** -20)
INV_QDIV = 1.0 / QDIV
TINY = 1e-30

M, K, N = 8192, 4096, 4096
MG, NG = 2, 4
M_loc, N_loc = M // MG, N // NG
N_CORES = MG * NG

HEAD_MIS = 5


def build_aqt(nc, M_loc, K, N_loc, W=512):
    KT, MT = K // P, M_loc // P
    NB = N_loc // W
    NT = N_loc // P
    TPB = NT // NB
    H = K // 2
    HT = H // P

    lhs = nc.declare_dram_parameter("lhs", [M_loc, K], f32, isOutput=False)
    rhsT = nc.declare_dram_parameter("rhsT", [N_loc, K], f32, isOutput=False)
    out = nc.declare_dram_parameter("out", [M_loc, N_loc], f32, isOutput=True)

    with tile.TileContext(nc) as tc, ExitStack() as ctx:
        pool = lambda name, bufs: ctx.enter_context(tc.tile_pool(name=name, bufs=bufs))
        qr_pool = pool("qr", 1)
        raws = pool("raws", 8)
        rqf = pool("rqf", 3)
        rsc = pool("rsc", 4)
        lqb = pool("lqb", 2)
        lqt = pool("lqt", 5)
        lsc = pool("lsc", 1)
        sml = pool("sml", 8)
        opool = pool("o1", 3)
        psum = ctx.enter_context(tc.tile_pool(name="psum", bufs=6, space="PSUM"))

        s_l_all = lsc.tile([P, MT], f32)
        qr_nb = [qr_pool.tile([P, KT, W], bf16, name=f"qrnb{nb}")
                 for nb in range(NB)]

        rraw_t, ram_t = {}, {}

        def emit_rhs_load(j):
            hs = []
            for h in range(2):
                raw = raws.tile([P, H], f32, name="raw")
                nc.gpsimd.dma_start(raw[:], rhsT[j * P:(j + 1) * P, h * H:(h + 1) * H])
                hs.append(raw)
            rraw_t[j] = hs

        def emit_rhs_red(j):
            ams = []
            for h in range(2):
                am = sml.tile([P, 1], f32, name="ram")
                nc.vector.tensor_reduce(am[:], rraw_t[j][h][:],
                                        axis=mybir.AxisListType.X,
                                        op=Alu.max, apply_absolute_value=True)
                ams.append(am)
            ram_t[j] = ams

        def emit_rhs_quant(j):
            nb, jo = divmod(j, TPB)
            hs = rraw_t.pop(j)
            am0, am1 = ram_t.pop(j)
            nc.vector.tensor_tensor(am0[:], am0[:], am1[:], op=Alu.max)
            s_col = rsc.tile([P, 1], f32, name="rs")
            nc.vector.tensor_scalar(s_col[:], am0[:], TINY, INV_QDIV,
                                    op0=Alu.max, op1=Alu.mult)
            r_col = sml.tile([P, 1], f32, name="rr")
            nc.vector.reciprocal(r_col[:], s_col[:])
            for h in range(2):
                raw = hs[h]
                nc.scalar.activation(raw[:], raw[:], Act.Copy,
                                     bias=C_MAGIC, scale=r_col[:])
                qf = rqf.tile([P, H], bf16, name="rqf")
                nc.vector.tensor_scalar(qf[:], raw[:], C_MAGIC, s_col[:],
                                        op0=Alu.subtract, op1=Alu.mult)
                nc.sync.dma_start_transpose(
                    qr_nb[nb][:, h * HT:(h + 1) * HT, jo * P:(jo + 1) * P], qf[:])

        lraw_t, lqt_t = {}, {}

        def emit_lhs_load(mi):
            rs = slice(mi * P, (mi + 1) * P)
            hs = []
            for h in range(2):
                raw = raws.tile([P, H], f32, name="raw")
                nc.gpsimd.dma_start(raw[:], lhs[rs, h * H:(h + 1) * H])
                hs.append(raw)
            lraw_t[mi] = hs

        def lq(mi):
            hs = lraw_t.pop(mi)
            ams = []
            for h in range(2):
                am = sml.tile([P, 1], f32, name="lam")
                nc.vector.tensor_reduce(am[:], hs[h][:], axis=mybir.AxisListType.X,
                                        op=Alu.max, apply_absolute_value=True)
                ams.append(am)
            am0, am1 = ams
            nc.vector.tensor_tensor(am0[:], am0[:], am1[:], op=Alu.max)
            s_col = s_l_all[:, mi:mi + 1]
            nc.vector.tensor_scalar(s_col, am0[:], TINY, INV_QDIV,
                                    op0=Alu.max, op1=Alu.mult)
            r_col = sml.tile([P, 1], f32, name="lr")
            nc.vector.reciprocal(r_col[:], s_col)
            qb = lqb.tile([P, K], bf16, name="lqb")
            for h in range(2):
                raw = hs[h]
                if mi >= 8:
                    nc.vector.tensor_scalar(raw[:], raw[:], r_col[:], C_MAGIC,
                                            op0=Alu.mult, op1=Alu.add)
                else:
                    nc.scalar.activation(raw[:], raw[:], Act.Copy,
                                         bias=C_MAGIC, scale=r_col[:])
                nc.vector.tensor_scalar(qb[:, h * H:(h + 1) * H], raw[:],
                                        C_MAGIC, None, op0=Alu.subtract)
            qt = lqt.tile([P, KT, P], bf16, name="lqt")
            nc.sync.dma_start_transpose(qt[:, :, :], qb[:, :])
            lqt_t[mi] = qt

        def emit_mm_group(mi, nb, last):
            qt = lqt_t[mi]
            if last:
                del lqt_t[mi]
            rs = slice(mi * P, (mi + 1) * P)
            ps = psum.tile([P, W], f32, name="ps")
            for kt in range(KT):
                nc.tensor.matmul(ps[:], qt[:, kt, :], qr_nb[nb][:, kt, :],
                                 start=(kt == 0), stop=(kt == KT - 1))
            o1 = opool.tile([P, W], f32, name="o1")
            nc.scalar.activation(o1[:], ps[:], Act.Copy, bias=0.0,
                                 scale=s_l_all[:, mi:mi + 1])
            nc.sync.dma_start(out[rs, nb * W:(nb + 1) * W], o1[:])

        emit_lhs_load(0)
        emit_rhs_load(0)
        emit_rhs_load(1)
        emit_rhs_load(2)
        emit_rhs_red(0)
        emit_rhs_quant(0)
        lq(0)
        emit_rhs_load(3)
        emit_rhs_red(1)
        emit_rhs_quant(1)
        emit_lhs_load(1)
        emit_rhs_red(2)
        emit_rhs_quant(2)
        emit_lhs_load(2)
        emit_rhs_load(4)
        emit_rhs_red(3)
        emit_rhs_quant(3)
        lq(1)
        emit_rhs_load(5)
        lq(2)
        emit_lhs_load(3)
        emit_rhs_red(4)
        emit_rhs_quant(4)
        emit_lhs_load(4)
        emit_rhs_load(6)
        lq(3)
        emit_rhs_red(5)
        emit_rhs_quant(5)
        emit_rhs_load(7)
        lq(4)
        emit_rhs_red(6)
        emit_rhs_quant(6)
        emit_lhs_load(5)
        emit_rhs_red(7)
        emit_rhs_quant(7)
        emit_lhs_load(6)
        for mi in range(HEAD_MIS):
            emit_mm_group(mi, 0, last=False)
        emit_mm_group(0, 1, last=True)
        lq(5)
        emit_lhs_load(7)
        emit_mm_group(1, 1, last=True)
        lq(6)
        emit_lhs_load(8)
        for mi in range(2, HEAD_MIS):
            emit_mm_group(mi, 1, last=True)

        for mi in range(HEAD_MIS, MT):
            emit_mm_group(mi, 0, last=False)
            if mi + 4 < MT:
                emit_lhs_load(mi + 4)
            if mi + 2 < MT:
                lq(mi + 2)
            emit_mm_group(mi, 1, last=True)
    return nc


_COMPILED_NC = None


def _get_compiled():
    global _COMPILED_NC
    if _COMPILED_NC is None:
        nc = bacc.Bacc("TRN2", target_bir_lowering=False, debug=False,
                       num_devices=N_CORES)
        build_aqt(nc, M_loc, K, N_loc)
        nc.compile()
        _COMPILED_NC = nc
    return _COMPILED_NC


def _shard(lhs, rhs):
    rhsT = np.ascontiguousarray(rhs.T)
    in_maps = []
    for i in range(N_CORES):
        mg, ng = divmod(i, NG)
        in_maps.append({
            "lhs": np.ascontiguousarray(lhs[mg * M_loc:(mg + 1) * M_loc, :]),
            "rhsT": rhsT[ng * N_loc:(ng + 1) * N_loc, :],
        })
    return in_maps


def kernel(lhs, rhs, _trace=False, _trace_kwargs=None):
    lhs = np.asarray(lhs, np.float32)
    rhs = np.asarray(rhs, np.float32)
    nc = _get_compiled()
    res = run_bass_kernel_spmd(nc, _shard(lhs, rhs), core_ids=list(range(N_CORES)),
                               trace=_trace, **(_trace_kwargs or {}))
    out = np.empty((M, N), np.float32)
    for i in range(N_CORES):
        mg, ng = divmod(i, NG)
        out[mg * M_loc:(mg + 1) * M_loc, ng * N_loc:(ng + 1) * N_loc] = \
            res.results[i]["out"]
    kernel.last_result = res
    return out
